# revision 19
# baseline (speedup 1.0000x reference)
"""AttentivePool Trainium2 kernel.

Reference computation per sample (x [C,T], prefix mask m [T]):
  stats1: mu/sd of x over valid frames
  h = w1a@x + (w1b@mu + w1c@sd + b1)  -> LayerNorm over 128 ch -> relu -> tanh
  a = w2@th (+b2), softmax over valid t, weighted mu2/sd2 of x -> [2C]

Sharding: pure data parallel, 2 samples per core across 8 cores.

Per-core layout strategy (natural [C-partitions, T-free]):
  - x resident in SBUF as 12 tiles [128, 2000] f32
  - T-reductions (masked moments, softmax denom, weighted moments) via DVE
    tensor_tensor_reduce / ACT accum_out -- no transposes of x
  - matmuls on PE in float32r (full-rate fp32 path, N>=256)
  - channel-LN done in transposed h-space (h is only [128, T]): PE transpose,
    bn_stats over free dim, normalize, transpose back
  - softmax max-subtraction skipped: |a| <~ 1 for this problem's scales, and
    masked lanes get -30 added via a PSUM preload before the mm2 accumulation
"""

import numpy as np

B, C, T = 16, 1536, 2000
ATTN = 128
NCORES = 8
BPC = B // NCORES          # samples per core
NCH = C // 128             # 12 channel chunks
TPAD = 2048                # padded T for 16x128 transpose blocks
NTT = TPAD // 128          # 16 T blocks
BANKS = [(0, 512), (512, 512), (1024, 512), (1536, 464)]  # bank-aligned T slices
MASK_NEG = -30.0

_CACHE = {}


def _split_waits(nc, max_waits=1):
    """walrus in this toolchain rejects >1 sync-wait per instruction; hoist
    excess waits onto injected same-engine NOPs that run just before."""
    from concourse import mybir
    ctr = 0
    for fn in nc.m.functions:
        for blk in fn.blocks:
            out = []
            changed = False
            for ins in blk.instructions:
                si = ins.sync_info
                ow = list(si.on_wait) if si and si.on_wait else []
                if len(ow) > max_waits:
                    changed = True
                    excess = ow[:-max_waits]
                    for i in range(0, len(excess), max_waits):
                        ctr += 1
                        out.append(mybir.InstNoOp(
                            name=f"wsplit_{ctr}", engine=ins.engine,
                            ins=[], outs=[],
                            sync_info=mybir.SyncInfo(
                                on_wait=excess[i:i + max_waits],
                                on_update=[])))
                    si.on_wait = ow[-max_waits:]
                    ins.sync_info = si
                out.append(ins)
            if changed:
                blk.instructions = out


def _build_nc(trivial_ln=True, trivial_b2=True):
    import concourse.bass as bass
    import concourse.tile as tile
    from concourse import mybir

    fp32 = mybir.dt.float32
    f32r = mybir.dt.float32r
    AF = mybir.ActivationFunctionType
    OP = mybir.AluOpType

    nc = bass.Bass("TRN2", target_bir_lowering=False)

    bf16 = mybir.dt.bfloat16
    x_d = nc.dram_tensor("x", [BPC, C, T], bf16, kind="ExternalInput")
    mk_d = nc.dram_tensor("maskb", [BPC, T], bf16, kind="ExternalInput")
    mbias_d = nc.dram_tensor("mbias", [BPC, T], bf16, kind="ExternalInput")
    w1aT_d = nc.dram_tensor("w1aT", [128, NCH, 128], bf16, kind="ExternalInput")
    w1bT_d = nc.dram_tensor("w1bT", [128, NCH, 128], fp32, kind="ExternalInput")
    w1cT_d = nc.dram_tensor("w1cT", [128, NCH, 128], fp32, kind="ExternalInput")
    w2T_d = nc.dram_tensor("w2T", [128, NCH, 128], bf16, kind="ExternalInput")
    gb_d = nc.dram_tensor("gb", [128, 128], fp32, kind="ExternalInput")
    bb_d = nc.dram_tensor("bb", [128, 128], fp32, kind="ExternalInput")
    b1_d = nc.dram_tensor("b1c", [128, 1], fp32, kind="ExternalInput")
    b2_d = nc.dram_tensor("b2c", [128, NCH], fp32, kind="ExternalInput")
    ones_d = nc.dram_tensor("ones_row", [1, 128], bf16, kind="ExternalInput")
    id_d = nc.dram_tensor("ident", [128, 128], fp32, kind="ExternalInput")
    out_d = nc.dram_tensor("out", [BPC, 2 * C], fp32, kind="ExternalOutput")

    with tile.TileContext(nc) as tc:
        import contextlib
        with contextlib.ExitStack() as ctx:
            consts = ctx.enter_context(tc.tile_pool(name="consts", bufs=1))
            xpool = ctx.enter_context(tc.tile_pool(name="xres", bufs=1))
            nmpool = ctx.enter_context(tc.tile_pool(name="nm", bufs=2))
            scr2 = ctx.enter_context(tc.tile_pool(name="scr2", bufs=2))
            scr1 = ctx.enter_context(tc.tile_pool(name="scr1", bufs=1))
            hpool = ctx.enter_context(tc.tile_pool(name="hbuf", bufs=2))
            stpool = ctx.enter_context(tc.tile_pool(name="stats", bufs=2))
            psum = ctx.enter_context(tc.tile_pool(name="ps", bufs=2, space="PSUM"))

            # --- constants (loaded once) ---
            w1aT = consts.tile([128, NCH, 128], bf16)
            w1bT = consts.tile([128, NCH, 128], fp32)
            w1cT = consts.tile([128, NCH, 128], fp32)
            w2T = consts.tile([128, NCH, 128], bf16)
            gb = consts.tile([128, 128], fp32)
            bb = consts.tile([128, 128], fp32)
            b1c = consts.tile([128, 1], fp32)
            b2c = consts.tile([128, NCH], fp32)
            ones_row = consts.tile([1, 128], bf16)
            ident = consts.tile([128, 128], fp32)
            eps_c = consts.tile([128, 1], fp32)
            nc.vector.memset(eps_c, 1e-5)
            for t_, d_ in ((w1aT, w1aT_d), (w1bT, w1bT_d), (w1cT, w1cT_d),
                           (w2T, w2T_d), (gb, gb_d), (bb, bb_d), (b1c, b1_d),
                           (b2c, b2_d), (ones_row, ones_d), (ident, id_d)):
                nc.sync.dma_start(out=t_, in_=d_[:])

            for s in range(BPC):
                # ---------- mask prep ----------
                mrow = stpool.tile([1, T], fp32, tag="mrow")
                nc.sync.dma_start(out=mrow, in_=mk_d[s, :][None, :])
                trow = stpool.tile([1, T], fp32, tag="trow")
                msum = stpool.tile([1, 1], fp32, tag="msum")
                nc.scalar.activation(out=trow, in_=mrow, func=AF.Copy,
                                     accum_out=msum)
                rcpL = stpool.tile([1, 1], fp32, tag="rcpL")
                nc.vector.reciprocal(out=rcpL, in_=msum)
                nm_row = stpool.tile([1, T], fp32, tag="nmrow")
                nc.vector.tensor_scalar(out=nm_row, in0=mrow, scalar1=rcpL,
                                        scalar2=None, op0=OP.mult)
                mb_row = stpool.tile([1, T], fp32, tag="mbrow")
                nc.vector.tensor_scalar(out=mb_row, in0=mrow, scalar1=1.0,
                                        scalar2=-MASK_NEG, op0=OP.subtract,
                                        op1=OP.mult)
                # broadcast nm and L across partitions
                nm_b = nmpool.tile([128, T], fp32, tag="nmb")
                nc.sync.dma_start(out=nm_b, in_=nm_row.to_broadcast((128, T)))
                L_b = stpool.tile([128, 1], fp32, tag="Lb")
                nc.sync.dma_start(out=L_b, in_=msum.to_broadcast((128, 1)))

                # per-sample stat vectors [128, NCH]
                sv = stpool.tile([128, 16 * NCH], fp32, tag="sv")
                mu_a = sv[:, 0 * NCH:1 * NCH]
                q_a = sv[:, 1 * NCH:2 * NCH]
                sd_a = sv[:, 2 * NCH:3 * NCH]
                den_a = sv[:, 3 * NCH:4 * NCH]
                sex_a = sv[:, 4 * NCH:5 * NCH]
                sx2_a = sv[:, 5 * NCH:6 * NCH]
                var_a = sv[:, 6 * NCH:7 * NCH]
                t0_a = sv[:, 7 * NCH:8 * NCH]
                mu2_a = sv[:, 8 * NCH:9 * NCH]
                ms2_a = sv[:, 9 * NCH:10 * NCH]
                sd2_a = sv[:, 10 * NCH:11 * NCH]
                rden_a = sv[:, 11 * NCH:12 * NCH]

                # ---------- phase 1: load x, mm1, masked moments ----------
                h_ps = psum.tile([128, TPAD], fp32, tag="mm")
                xt = []
                for i in range(NCH):
                    xb = xpool.tile([128, T], bf16, tag=f"x{i}")
                    xt.append(xb)
                    nc.sync.dma_start(out=xb, in_=x_d[s, i * 128:(i + 1) * 128, :])
                    for (o, n) in BANKS:
                        nc.tensor.matmul(
                            h_ps[:, o:o + n],
                            lhsT=w1aT[:, i, :],
                            rhs=xb[:, o:o + n],
                            start=(i == 0), stop=(i == NCH - 1))
                    xnm = scr2.tile([128, T], fp32, tag="xnm")
                    nc.vector.tensor_tensor_reduce(
                        out=xnm, in0=xi.bitcast(fp32), in1=nm_b, scale=1.0,
                        scalar=0.0,
                        op0=OP.mult, op1=OP.add, accum_out=mu_a[:, i:i + 1])
                    nc.scalar.activation(out=xnm, in_=xnm, func=AF.Square,
                                         accum_out=q_a[:, i:i + 1])

                # ---------- phase 2: mu/sd, column bias cb ----------
                # E[x^2] = L * sum((nm*x)^2) ; var = E[x^2] - mu^2
                nc.vector.tensor_scalar(out=q_a, in0=q_a, scalar1=L_b,
                                        scalar2=None, op0=OP.mult)
                nc.vector.tensor_mul(out=t0_a, in0=mu_a, in1=mu_a)
                nc.vector.tensor_tensor(out=var_a, in0=q_a, in1=t0_a,
                                        op=OP.subtract)
                nc.vector.tensor_scalar(out=var_a, in0=var_a, scalar1=1e-9,
                                        scalar2=None, op0=OP.max)
                nc.scalar.activation(out=sd_a, in_=var_a, func=AF.Sqrt)

                cb_ps = psum.tile([128, 1], fp32, tag="mm")
                for i in range(NCH):
                    nc.tensor.matmul(cb_ps, lhsT=w1bT[:, i, :],
                                     rhs=mu_a[:, i:i + 1],
                                     start=(i == 0), stop=False)
                for i in range(NCH):
                    nc.tensor.matmul(cb_ps, lhsT=w1cT[:, i, :],
                                     rhs=sd_a[:, i:i + 1],
                                     start=False, stop=(i == NCH - 1))
                cb = stpool.tile([128, 1], fp32, tag="cb")
                nc.scalar.activation(out=cb, in_=cb_ps, func=AF.Identity, bias=b1c)

                # ---------- phase 3: h -> LN -> relu -> tanh -> th ----------
                h_sb = hpool.tile([128, TPAD], fp32, tag="hbuf")
                nc.scalar.activation(out=h_sb[:, 0:T], in_=h_ps[:, 0:T],
                                     func=AF.Identity, bias=cb)
                nc.vector.memset(h_sb[:, T:TPAD], 0.0)

                tp_ps = psum.tile([128, TPAD], fp32, tag="mm")
                for j in range(NTT):
                    nc.tensor.transpose(tp_ps[:, j * 128:(j + 1) * 128],
                                        in_=h_sb[:, j * 128:(j + 1) * 128],
                                        identity=ident)
                hT = hpool.tile([128, TPAD], fp32, tag="hbuf")
                for g in range(4):
                    nc.scalar.activation(out=hT[:, g * 512:(g + 1) * 512],
                                         in_=tp_ps[:, g * 512:(g + 1) * 512],
                                         func=AF.Copy)

                st6 = stpool.tile([128, NTT, 6], fp32, tag="st6")
                mv = stpool.tile([128, NTT, 2], fp32, tag="mv")
                for j in range(NTT):
                    nc.vector.bn_stats(out=st6[:, j, :],
                                       in_=hT[:, j * 128:(j + 1) * 128])
                    nc.vector.bn_aggr(out=mv[:, j, :], in_=st6[:, j, :])
                rstd = stpool.tile([128, NTT], fp32, tag="rstd")
                nc.scalar.activation(out=rstd, in_=mv[:, :, 1], func=AF.Sqrt,
                                     bias=eps_c)
                nc.vector.reciprocal(out=rstd, in_=rstd)

                thT = hpool.tile([128, TPAD], fp32, tag="hbuf")
                for j in range(NTT):
                    blk = slice(j * 128, (j + 1) * 128)
                    nc.vector.tensor_scalar(
                        out=thT[:, blk], in0=hT[:, blk],
                        scalar1=mv[:, j, 0:1], scalar2=rstd[:, j:j + 1],
                        op0=OP.subtract, op1=OP.mult)
                    if not trivial_ln:
                        nc.vector.tensor_mul(out=thT[:, blk],
                                             in0=thT[:, blk], in1=gb)
                        nc.vector.tensor_add(out=thT[:, blk],
                                             in0=thT[:, blk], in1=bb)
                nc.vector.tensor_scalar(out=thT, in0=thT, scalar1=0.0,
                                        scalar2=None, op0=OP.max)
                nc.scalar.activation(out=thT, in_=thT, func=AF.Tanh)

                tb_ps = psum.tile([128, TPAD], fp32, tag="mm")
                for j in range(NTT):
                    nc.tensor.transpose(tb_ps[:, j * 128:(j + 1) * 128],
                                        in_=thT[:, j * 128:(j + 1) * 128],
                                        identity=ident)
                th = hpool.tile([128, TPAD], bf16, tag="thbuf")
                for g in range(4):
                    nc.scalar.activation(out=th[:, g * 512:(g + 1) * 512],
                                         in_=tb_ps[:, g * 512:(g + 1) * 512],
                                         func=AF.Copy)

                # ---------- phase 4: mm2 + exp + weighted moments ----------
                for i in range(NCH):
                    a_ps = psum.tile([128, TPAD], fp32, tag="mm")
                    for (o, n) in BANKS:
                        nc.tensor.matmul(
                            a_ps[:, o:o + n],
                            lhsT=ones_row,
                            rhs=mb_row[:, o:o + n],
                            start=True, stop=False)
                        nc.tensor.matmul(
                            a_ps[:, o:o + n],
                            lhsT=w2T[:, i, :],
                            rhs=th[:, o:o + n],
                            start=False, stop=True)
                    e_i = scr2.tile([128, T], fp32, tag="ei")
                    nc.scalar.activation(out=e_i, in_=a_ps[:, 0:T], func=AF.Exp,
                                         bias=b2c[:, i:i + 1],
                                         accum_out=den_a[:, i:i + 1])
                    ex_i = scr1.tile([128, T], fp32, tag="exi")
                    nc.vector.tensor_tensor_reduce(
                        out=ex_i, in0=e_i, in1=xt[i].bitcast(fp32), scale=1.0,
                        scalar=0.0,
                        op0=OP.mult, op1=OP.add, accum_out=sex_a[:, i:i + 1])
                    nc.vector.tensor_tensor_reduce(
                        out=ex_i, in0=ex_i, in1=xt[i].bitcast(fp32), scale=1.0,
                        scalar=0.0,
                        op0=OP.mult, op1=OP.add, accum_out=sx2_a[:, i:i + 1])

                # ---------- phase 5: outputs ----------
                nc.vector.reciprocal(out=rden_a, in_=den_a)
                nc.vector.tensor_mul(out=mu2_a, in0=sex_a, in1=rden_a)
                nc.vector.tensor_mul(out=ms2_a, in0=sx2_a, in1=rden_a)
                nc.vector.tensor_mul(out=t0_a, in0=mu2_a, in1=mu2_a)
                nc.vector.tensor_tensor(out=ms2_a, in0=ms2_a, in1=t0_a,
                                        op=OP.subtract)
                nc.vector.tensor_scalar(out=ms2_a, in0=ms2_a, scalar1=1e-9,
                                        scalar2=None, op0=OP.max)
                nc.scalar.activation(out=sd2_a, in_=ms2_a, func=AF.Sqrt)

                nc.sync.dma_start(
                    out=out_d[s, 0:C].rearrange("(i p) -> p i", p=128),
                    in_=mu2_a)
                nc.sync.dma_start(
                    out=out_d[s, C:2 * C].rearrange("(i p) -> p i", p=128),
                    in_=sd2_a)

    _split_waits(nc)
    return nc


def _prep_weights(w1, b1, ln_g, ln_b, w2, b2):
    f = np.float32
    w1T = np.ascontiguousarray(w1.T, dtype=f)            # [3C, 128]
    import ml_dtypes as _md
    w1aT = np.ascontiguousarray(
        w1T[0:C].reshape(NCH, 128, 128).transpose(1, 0, 2)).astype(_md.bfloat16)
    w1bT = np.ascontiguousarray(
        w1T[C:2 * C].reshape(NCH, 128, 128).transpose(1, 0, 2))
    w1cT = np.ascontiguousarray(
        w1T[2 * C:3 * C].reshape(NCH, 128, 128).transpose(1, 0, 2))
    import ml_dtypes
    bf = ml_dtypes.bfloat16
    w2T = np.ascontiguousarray(
        np.asarray(w2, f).reshape(NCH, 128, 128).transpose(2, 0, 1)).astype(bf)
    gb = np.ascontiguousarray(np.tile(np.asarray(ln_g, f)[None, :], (128, 1)))
    bb = np.ascontiguousarray(np.tile(np.asarray(ln_b, f)[None, :], (128, 1)))
    b1c = np.asarray(b1, f).reshape(128, 1).copy()
    b2c = np.ascontiguousarray(np.asarray(b2, f).reshape(NCH, 128).T)
    ones_row = np.ones((1, 128), bf)
    ident = np.eye(128, dtype=f)
    return dict(w1aT=w1aT, w1bT=w1bT, w1cT=w1cT, w2T=w2T, gb=gb, bb=bb,
                b1c=b1c, b2c=b2c, ones_row=ones_row, ident=ident)


def kernel(x, mask, w1, b1, ln_g, ln_b, w2, b2, _profile=None):
    from concourse.bass_utils import run_bass_kernel_spmd

    trivial_ln = bool(np.all(np.asarray(ln_g) == 1.0)
                      and np.all(np.asarray(ln_b) == 0.0))
    trivial_b2 = bool(np.all(np.asarray(b2) == 0.0))
    key = ("nc", trivial_ln, trivial_b2)
    if key not in _CACHE:
        _CACHE[key] = _build_nc(trivial_ln, trivial_b2)
    nc = _CACHE[key]

    wts = _prep_weights(w1, b1, ln_g, ln_b, w2, b2)
    import ml_dtypes
    xf = np.ascontiguousarray(
        np.asarray(x, np.float32).astype(ml_dtypes.bfloat16))
    mf = np.ascontiguousarray(np.asarray(mask, np.float32).reshape(B, T))
    maskb = np.ascontiguousarray(mf.astype(ml_dtypes.bfloat16))
    mbias = np.ascontiguousarray(
        ((mf - 1.0) * 30.0).astype(ml_dtypes.bfloat16))

    in_maps = []
    for c in range(NCORES):
        m = {"x": xf[c * BPC:(c + 1) * BPC],
             "maskb": maskb[c * BPC:(c + 1) * BPC],
             "mbias": mbias[c * BPC:(c + 1) * BPC]}
        m.update(wts)
        in_maps.append(m)

    kw = dict(_profile) if _profile else {}
    res = run_bass_kernel_spmd(nc, in_maps, list(range(NCORES)), **kw)
    out = np.concatenate([res.results[c]["out"] for c in range(NCORES)], axis=0)
    if _profile:
        _CACHE["last_result"] = res
    return out.reshape(B, 2 * C)


# revision 20
# speedup vs baseline: 1.0300x; 1.0300x over previous
"""AttentivePool Trainium2 kernel.

Reference computation per sample (x [C,T], prefix mask m [T]):
  stats1: mu/sd of x over valid frames
  h = w1a@x + (w1b@mu + w1c@sd + b1)  -> LayerNorm over 128 ch -> relu -> tanh
  a = w2@th (+b2), softmax over valid t, weighted mu2/sd2 of x -> [2C]

Sharding: pure data parallel, 2 samples per core across 8 cores.

Per-core layout strategy (natural [C-partitions, T-free]):
  - x resident in SBUF as 12 tiles [128, 2000] f32
  - T-reductions (masked moments, softmax denom, weighted moments) via DVE
    tensor_tensor_reduce / ACT accum_out -- no transposes of x
  - matmuls on PE in float32r (full-rate fp32 path, N>=256)
  - channel-LN done in transposed h-space (h is only [128, T]): PE transpose,
    bn_stats over free dim, normalize, transpose back
  - softmax max-subtraction skipped: |a| <~ 1 for this problem's scales, and
    masked lanes get -30 added via a PSUM preload before the mm2 accumulation
"""

import numpy as np

B, C, T = 16, 1536, 2000
ATTN = 128
NCORES = 8
BPC = B // NCORES          # samples per core
NCH = C // 128             # 12 channel chunks
TPAD = 2048                # padded T for 16x128 transpose blocks
NTT = TPAD // 128          # 16 T blocks
BANKS = [(0, 512), (512, 512), (1024, 512), (1536, 464)]  # bank-aligned T slices
MASK_NEG = -30.0

_CACHE = {}


def _split_waits(nc, max_waits=1):
    """walrus in this toolchain rejects >1 sync-wait per instruction; hoist
    excess waits onto injected same-engine NOPs that run just before."""
    from concourse import mybir
    ctr = 0
    for fn in nc.m.functions:
        for blk in fn.blocks:
            out = []
            changed = False
            for ins in blk.instructions:
                si = ins.sync_info
                ow = list(si.on_wait) if si and si.on_wait else []
                if len(ow) > max_waits:
                    changed = True
                    excess = ow[:-max_waits]
                    for i in range(0, len(excess), max_waits):
                        ctr += 1
                        out.append(mybir.InstNoOp(
                            name=f"wsplit_{ctr}", engine=ins.engine,
                            ins=[], outs=[],
                            sync_info=mybir.SyncInfo(
                                on_wait=excess[i:i + max_waits],
                                on_update=[])))
                    si.on_wait = ow[-max_waits:]
                    ins.sync_info = si
                out.append(ins)
            if changed:
                blk.instructions = out


def _build_nc(trivial_ln=True, trivial_b2=True):
    import concourse.bass as bass
    import concourse.tile as tile
    from concourse import mybir

    fp32 = mybir.dt.float32
    f32r = mybir.dt.float32r
    AF = mybir.ActivationFunctionType
    OP = mybir.AluOpType

    nc = bass.Bass("TRN2", target_bir_lowering=False)

    bf16 = mybir.dt.bfloat16
    x_d = nc.dram_tensor("x", [BPC, C, T], bf16, kind="ExternalInput")
    mk_d = nc.dram_tensor("maskb", [BPC, T], bf16, kind="ExternalInput")
    mbias_d = nc.dram_tensor("mbias", [BPC, T], bf16, kind="ExternalInput")
    w1aT_d = nc.dram_tensor("w1aT", [128, NCH, 128], bf16, kind="ExternalInput")
    w1bT_d = nc.dram_tensor("w1bT", [128, NCH, 128], fp32, kind="ExternalInput")
    w1cT_d = nc.dram_tensor("w1cT", [128, NCH, 128], fp32, kind="ExternalInput")
    w2T_d = nc.dram_tensor("w2T", [128, NCH, 128], bf16, kind="ExternalInput")
    gb_d = nc.dram_tensor("gb", [128, 128], fp32, kind="ExternalInput")
    bb_d = nc.dram_tensor("bb", [128, 128], fp32, kind="ExternalInput")
    b1_d = nc.dram_tensor("b1c", [128, 1], fp32, kind="ExternalInput")
    b2_d = nc.dram_tensor("b2c", [128, NCH], fp32, kind="ExternalInput")
    ones_d = nc.dram_tensor("ones_row", [1, 128], bf16, kind="ExternalInput")
    id_d = nc.dram_tensor("ident", [128, 128], fp32, kind="ExternalInput")
    out_d = nc.dram_tensor("out", [BPC, 2 * C], fp32, kind="ExternalOutput")

    with tile.TileContext(nc) as tc:
        import contextlib
        with contextlib.ExitStack() as ctx:
            consts = ctx.enter_context(tc.tile_pool(name="consts", bufs=1))
            xpool = ctx.enter_context(tc.tile_pool(name="xres", bufs=1))
            nmpool = ctx.enter_context(tc.tile_pool(name="nm", bufs=2))
            scr2 = ctx.enter_context(tc.tile_pool(name="scr2", bufs=2))
            scr1 = ctx.enter_context(tc.tile_pool(name="scr1", bufs=1))
            hpool = ctx.enter_context(tc.tile_pool(name="hbuf", bufs=2))
            stpool = ctx.enter_context(tc.tile_pool(name="stats", bufs=2))
            psum = ctx.enter_context(tc.tile_pool(name="ps", bufs=1, space="PSUM"))
            psa = ctx.enter_context(tc.tile_pool(name="psa", bufs=2, space="PSUM"))

            # --- constants (loaded once) ---
            w1aT = consts.tile([128, NCH, 128], bf16)
            w1bT = consts.tile([128, NCH, 128], fp32)
            w1cT = consts.tile([128, NCH, 128], fp32)
            w2T = consts.tile([128, NCH, 128], bf16)
            gb = consts.tile([128, 128], fp32)
            bb = consts.tile([128, 128], fp32)
            b1c = consts.tile([128, 1], fp32)
            b2c = consts.tile([128, NCH], fp32)
            ones_row = consts.tile([1, 128], bf16)
            ident = consts.tile([128, 128], fp32)
            eps_c = consts.tile([128, 1], fp32)
            nc.vector.memset(eps_c, 1e-5)
            for t_, d_ in ((w1aT, w1aT_d), (w1bT, w1bT_d), (w1cT, w1cT_d),
                           (w2T, w2T_d), (gb, gb_d), (bb, bb_d), (b1c, b1_d),
                           (b2c, b2_d), (ones_row, ones_d), (ident, id_d)):
                nc.sync.dma_start(out=t_, in_=d_[:])

            for s in range(BPC):
                # ---------- mask prep ----------
                mrow = stpool.tile([1, T], fp32, tag="mrow")
                nc.sync.dma_start(out=mrow, in_=mk_d[s, :][None, :])
                trow = stpool.tile([1, T], fp32, tag="trow")
                msum = stpool.tile([1, 1], fp32, tag="msum")
                nc.scalar.activation(out=trow, in_=mrow, func=AF.Copy,
                                     accum_out=msum)
                rcpL = stpool.tile([1, 1], fp32, tag="rcpL")
                nc.vector.reciprocal(out=rcpL, in_=msum)
                nm_row = stpool.tile([1, T], fp32, tag="nmrow")
                nc.vector.tensor_scalar(out=nm_row, in0=mrow, scalar1=rcpL,
                                        scalar2=None, op0=OP.mult)
                mb_row = stpool.tile([1, T], fp32, tag="mbrow")
                nc.vector.tensor_scalar(out=mb_row, in0=mrow, scalar1=1.0,
                                        scalar2=-MASK_NEG, op0=OP.subtract,
                                        op1=OP.mult)
                # broadcast nm and L across partitions
                nm_b = nmpool.tile([128, T], fp32, tag="nmb")
                nc.sync.dma_start(out=nm_b, in_=nm_row.to_broadcast((128, T)))
                L_b = stpool.tile([128, 1], fp32, tag="Lb")
                nc.sync.dma_start(out=L_b, in_=msum.to_broadcast((128, 1)))

                # per-sample stat vectors [128, NCH]
                sv = stpool.tile([128, 16 * NCH], fp32, tag="sv")
                mu_a = sv[:, 0 * NCH:1 * NCH]
                q_a = sv[:, 1 * NCH:2 * NCH]
                sd_a = sv[:, 2 * NCH:3 * NCH]
                den_a = sv[:, 3 * NCH:4 * NCH]
                sex_a = sv[:, 4 * NCH:5 * NCH]
                sx2_a = sv[:, 5 * NCH:6 * NCH]
                var_a = sv[:, 6 * NCH:7 * NCH]
                t0_a = sv[:, 7 * NCH:8 * NCH]
                mu2_a = sv[:, 8 * NCH:9 * NCH]
                ms2_a = sv[:, 9 * NCH:10 * NCH]
                sd2_a = sv[:, 10 * NCH:11 * NCH]
                rden_a = sv[:, 11 * NCH:12 * NCH]

                # ---------- phase 1: load x, mm1, masked moments ----------
                h_ps = psum.tile([128, TPAD], fp32, tag="mm")
                xt = []
                for i in range(NCH):
                    xb = xpool.tile([128, T], bf16, tag=f"x{i}")
                    xt.append(xb)
                    nc.sync.dma_start(out=xb, in_=x_d[s, i * 128:(i + 1) * 128, :])
                    for (o, n) in BANKS:
                        nc.tensor.matmul(
                            h_ps[:, o:o + n],
                            lhsT=w1aT[:, i, :],
                            rhs=xb[:, o:o + n],
                            start=(i == 0), stop=(i == NCH - 1))
                    xnm = scr2.tile([128, T], fp32, tag="xnm")
                    nc.vector.tensor_tensor_reduce(
                        out=xnm, in0=xi.bitcast(fp32), in1=nm_b, scale=1.0,
                        scalar=0.0,
                        op0=OP.mult, op1=OP.add, accum_out=mu_a[:, i:i + 1])
                    nc.scalar.activation(out=xnm, in_=xnm, func=AF.Square,
                                         accum_out=q_a[:, i:i + 1])

                # ---------- phase 2: mu/sd, column bias cb ----------
                # E[x^2] = L * sum((nm*x)^2) ; var = E[x^2] - mu^2
                nc.vector.tensor_scalar(out=q_a, in0=q_a, scalar1=L_b,
                                        scalar2=None, op0=OP.mult)
                nc.vector.tensor_mul(out=t0_a, in0=mu_a, in1=mu_a)
                nc.vector.tensor_tensor(out=var_a, in0=q_a, in1=t0_a,
                                        op=OP.subtract)
                nc.vector.tensor_scalar(out=var_a, in0=var_a, scalar1=1e-9,
                                        scalar2=None, op0=OP.max)
                nc.scalar.activation(out=sd_a, in_=var_a, func=AF.Sqrt)

                cb_ps = psa.tile([128, 1], fp32, tag="a")
                for i in range(NCH):
                    nc.tensor.matmul(cb_ps, lhsT=w1bT[:, i, :],
                                     rhs=mu_a[:, i:i + 1],
                                     start=(i == 0), stop=False)
                for i in range(NCH):
                    nc.tensor.matmul(cb_ps, lhsT=w1cT[:, i, :],
                                     rhs=sd_a[:, i:i + 1],
                                     start=False, stop=(i == NCH - 1))
                cb = stpool.tile([128, 1], fp32, tag="cb")
                nc.scalar.activation(out=cb, in_=cb_ps, func=AF.Identity, bias=b1c)

                # ---------- phase 3: h -> LN -> relu -> tanh -> th ----------
                h_sb = hpool.tile([128, TPAD], fp32, tag="hbuf")
                nc.scalar.activation(out=h_sb[:, 0:T], in_=h_ps[:, 0:T],
                                     func=AF.Identity, bias=cb)
                nc.vector.memset(h_sb[:, T:TPAD], 0.0)

                hT = hpool.tile([128, TPAD], fp32, tag="hbuf")
                for g in range(2):
                    tp_ps = psa.tile([128, 1024], fp32, tag="a")
                    for j in range(8):
                        jj = g * 8 + j
                        nc.tensor.transpose(tp_ps[:, j * 128:(j + 1) * 128],
                                            in_=h_sb[:, jj * 128:(jj + 1) * 128],
                                            identity=ident)
                    nc.scalar.activation(out=hT[:, g * 1024:(g + 1) * 1024],
                                         in_=tp_ps, func=AF.Copy)

                st6 = stpool.tile([128, NTT, 6], fp32, tag="st6")
                mv = stpool.tile([128, NTT, 2], fp32, tag="mv")
                for j in range(NTT):
                    nc.vector.bn_stats(out=st6[:, j, :],
                                       in_=hT[:, j * 128:(j + 1) * 128])
                    nc.vector.bn_aggr(out=mv[:, j, :], in_=st6[:, j, :])
                rstd = stpool.tile([128, NTT], fp32, tag="rstd")
                nc.scalar.activation(out=rstd, in_=mv[:, :, 1], func=AF.Sqrt,
                                     bias=eps_c)
                nc.vector.reciprocal(out=rstd, in_=rstd)

                thT = hpool.tile([128, TPAD], fp32, tag="hbuf")
                for j in range(NTT):
                    blk = slice(j * 128, (j + 1) * 128)
                    nc.vector.tensor_scalar(
                        out=thT[:, blk], in0=hT[:, blk],
                        scalar1=mv[:, j, 0:1], scalar2=rstd[:, j:j + 1],
                        op0=OP.subtract, op1=OP.mult)
                    if not trivial_ln:
                        nc.vector.tensor_mul(out=thT[:, blk],
                                             in0=thT[:, blk], in1=gb)
                        nc.vector.tensor_add(out=thT[:, blk],
                                             in0=thT[:, blk], in1=bb)
                nc.vector.tensor_scalar(out=thT, in0=thT, scalar1=0.0,
                                        scalar2=None, op0=OP.max)
                nc.scalar.activation(out=thT, in_=thT, func=AF.Tanh)

                th = hpool.tile([128, TPAD], bf16, tag="thbuf")
                for g in range(2):
                    tb_ps = psa.tile([128, 1024], fp32, tag="a")
                    for j in range(8):
                        jj = g * 8 + j
                        nc.tensor.transpose(tb_ps[:, j * 128:(j + 1) * 128],
                                            in_=thT[:, jj * 128:(jj + 1) * 128],
                                            identity=ident)
                    nc.scalar.activation(out=th[:, g * 1024:(g + 1) * 1024],
                                         in_=tb_ps, func=AF.Copy)

                # ---------- phase 4: mm2 + exp + weighted moments ----------
                for i in range(NCH):
                    a_ps = psum.tile([128, TPAD], fp32, tag="mm")
                    for (o, n) in BANKS:
                        nc.tensor.matmul(
                            a_ps[:, o:o + n],
                            lhsT=ones_row,
                            rhs=mb_row[:, o:o + n],
                            start=True, stop=False)
                        nc.tensor.matmul(
                            a_ps[:, o:o + n],
                            lhsT=w2T[:, i, :],
                            rhs=th[:, o:o + n],
                            start=False, stop=True)
                    e_i = scr2.tile([128, T], fp32, tag="ei")
                    nc.scalar.activation(out=e_i, in_=a_ps[:, 0:T], func=AF.Exp,
                                         bias=b2c[:, i:i + 1],
                                         accum_out=den_a[:, i:i + 1])
                    ex_i = scr1.tile([128, T], fp32, tag="exi")
                    nc.vector.tensor_tensor_reduce(
                        out=ex_i, in0=e_i, in1=xt[i].bitcast(fp32), scale=1.0,
                        scalar=0.0,
                        op0=OP.mult, op1=OP.add, accum_out=sex_a[:, i:i + 1])
                    nc.vector.tensor_tensor_reduce(
                        out=ex_i, in0=ex_i, in1=xt[i].bitcast(fp32), scale=1.0,
                        scalar=0.0,
                        op0=OP.mult, op1=OP.add, accum_out=sx2_a[:, i:i + 1])

                # ---------- phase 5: outputs ----------
                nc.vector.tensor_add(out=den_a, in0=denh[:, 0:NCH],
                                     in1=denh[:, NCH:2 * NCH])
                nc.vector.reciprocal(out=rden_a, in_=den_a)
                nc.vector.tensor_mul(out=mu2_a, in0=sex_a, in1=rden_a)
                nc.vector.tensor_mul(out=ms2_a, in0=sx2_a, in1=rden_a)
                nc.vector.tensor_mul(out=t0_a, in0=mu2_a, in1=mu2_a)
                nc.vector.tensor_tensor(out=ms2_a, in0=ms2_a, in1=t0_a,
                                        op=OP.subtract)
                nc.vector.tensor_scalar(out=ms2_a, in0=ms2_a, scalar1=1e-9,
                                        scalar2=None, op0=OP.max)
                nc.scalar.activation(out=sd2_a, in_=ms2_a, func=AF.Sqrt)

                nc.sync.dma_start(
                    out=out_d[s, 0:C].rearrange("(i p) -> p i", p=128),
                    in_=mu2_a)
                nc.sync.dma_start(
                    out=out_d[s, C:2 * C].rearrange("(i p) -> p i", p=128),
                    in_=sd2_a)

    _split_waits(nc)
    return nc


def _prep_weights(w1, b1, ln_g, ln_b, w2, b2):
    f = np.float32
    w1T = np.ascontiguousarray(w1.T, dtype=f)            # [3C, 128]
    import ml_dtypes as _md
    w1aT = np.ascontiguousarray(
        w1T[0:C].reshape(NCH, 128, 128).transpose(1, 0, 2)).astype(_md.bfloat16)
    w1bT = np.ascontiguousarray(
        w1T[C:2 * C].reshape(NCH, 128, 128).transpose(1, 0, 2))
    w1cT = np.ascontiguousarray(
        w1T[2 * C:3 * C].reshape(NCH, 128, 128).transpose(1, 0, 2))
    import ml_dtypes
    bf = ml_dtypes.bfloat16
    w2T = np.ascontiguousarray(
        np.asarray(w2, f).reshape(NCH, 128, 128).transpose(2, 0, 1)).astype(bf)
    gb = np.ascontiguousarray(np.tile(np.asarray(ln_g, f)[None, :], (128, 1)))
    bb = np.ascontiguousarray(np.tile(np.asarray(ln_b, f)[None, :], (128, 1)))
    b1c = np.asarray(b1, f).reshape(128, 1).copy()
    b2c = np.ascontiguousarray(np.asarray(b2, f).reshape(NCH, 128).T)
    ones_row = np.ones((1, 128), bf)
    ident = np.eye(128, dtype=f)
    return dict(w1aT=w1aT, w1bT=w1bT, w1cT=w1cT, w2T=w2T, gb=gb, bb=bb,
                b1c=b1c, b2c=b2c, ones_row=ones_row, ident=ident)


def kernel(x, mask, w1, b1, ln_g, ln_b, w2, b2, _profile=None):
    from concourse.bass_utils import run_bass_kernel_spmd

    trivial_ln = bool(np.all(np.asarray(ln_g) == 1.0)
                      and np.all(np.asarray(ln_b) == 0.0))
    trivial_b2 = bool(np.all(np.asarray(b2) == 0.0))
    key = ("nc", trivial_ln, trivial_b2)
    if key not in _CACHE:
        _CACHE[key] = _build_nc(trivial_ln, trivial_b2)
    nc = _CACHE[key]

    wts = _prep_weights(w1, b1, ln_g, ln_b, w2, b2)
    import ml_dtypes
    xf = np.ascontiguousarray(
        np.asarray(x, np.float32).astype(ml_dtypes.bfloat16))
    mf = np.ascontiguousarray(np.asarray(mask, np.float32).reshape(B, T))
    maskb = np.ascontiguousarray(mf.astype(ml_dtypes.bfloat16))
    mbias = np.ascontiguousarray(
        ((mf - 1.0) * 30.0).astype(ml_dtypes.bfloat16))

    in_maps = []
    for c in range(NCORES):
        m = {"x": xf[c * BPC:(c + 1) * BPC],
             "maskb": maskb[c * BPC:(c + 1) * BPC],
             "mbias": mbias[c * BPC:(c + 1) * BPC]}
        m.update(wts)
        in_maps.append(m)

    kw = dict(_profile) if _profile else {}
    res = run_bass_kernel_spmd(nc, in_maps, list(range(NCORES)), **kw)
    out = np.concatenate([res.results[c]["out"] for c in range(NCORES)], axis=0)
    if _profile:
        _CACHE["last_result"] = res
    return out.reshape(B, 2 * C)
